# revision 1
# baseline (speedup 1.0000x reference)
"""Trainium2 Bass kernel for nn_ContextProjector (moe_routing).

Reference computation:
    projected = split_heads(x @ W_x + b_x)            # (B,H,N,D)
    fx        = split_heads(x @ W_fx + b_fx)          # (B,H,N,D)
    sp        = projected @ W_slice + b_slice         # (B,H,N,S)
    w         = softmax(sp / clip(temp,.5,5))         # (B,H,N,S)
    norm      = w.sum(axis=N)                         # (B,H,S)
    out       = einsum('bhns,bhnd->bhsd', w/(norm+.01), fx)

Key algebraic restructuring (all exact):
  * projected is only used for sp, so fold on host:
        Wc[c,(h,s)] = sum_d W_x[c,(h,d)] W_slice[d,s] / t[h]
        bc[(h,s)]   = (b_x[h] @ W_slice + b_slice) / t[h]
    and sp/t = x @ Wc + bc.
  * fx never exists on device. With w~ the per-token softmax:
        sum_n w~[n,s] (x[n,:] @ W_fx + b_fx)[d]
          = (sum_n w~[n,s] [x[n,:] | 1]) @ [W_fx; b_fx]  =  G[s, :] @ ...
    so the device only accumulates G[(h,s), c] = sum_n w~[n,(h,s)] [x|1][n,c]
    into PSUM; the tiny G @ W_fx, the b_fx term, and the final divide by
    (norm+0.01) happen on host in float64. Column c=C of G is the norm.

Device per core (8 cores: core = 4*b + quarter-of-N, 16384 tokens each),
per 128-token subtile:
  PE : logits psum = bias-chunk + xT_k0 @ Wc_k0 + xT_k1 @ Wc_k1   (3 MMs)
  ACT: w = exp(logits psum) -> fp16 SBUF
  DVE: den = per-(token,head) sum over S; rec = 1/den
  GpS: w~ = w * rec (broadcast multiply; GpSimd's software op beats DVE's
       slow-path here and keeps DVE free)
  PE : per head-pair j: G_psum[j] += w~[:, pair]^T @ [x|1]  (4 MMs, N=257)
G matmuls are emitted 3 pairs behind their producers (software pipelining)
so the PE never stalls on the exp->reduce->normalize chain; 14 warm-up
matmuls at kernel start hold the PE HAM clock-gate at 8/8 through the
first DMA. All matmul operands fp16 (PSUM accumulates fp32).
"""

import numpy as np

import concourse.bass as bass
import concourse.mybir as mybir
import concourse.tile as tile
from concourse import bacc
from concourse.bass_utils import run_bass_kernel_spmd

# Problem shape (hardcoded per contract)
B, N, C = 2, 65536, 256
H, D, S = 8, 64, 64
HS = H * S    # 512
P = 128
NCORES = 8
SHARDS_PER_B = NCORES // B   # 4
T = N // SHARDS_PER_B        # 16384 tokens per core
CA = C + 1                   # token-major x augmented with a ones column

f16 = mybir.dt.float16
f32 = mybir.dt.float32


def _emit(ctx, tc, xt, wc, xtm, out, t_tokens, tt):
    nc = tc.nc
    KO = C // P              # 2 K-chunks of x
    n_blk = t_tokens // tt
    n_sub = tt // P          # subtiles (128 tokens) per block
    assert n_sub % 2 == 0

    consts = ctx.enter_context(tc.tile_pool(name="consts", bufs=1))
    xpool = ctx.enter_context(tc.tile_pool(name="xpool", bufs=3))
    mpool = ctx.enter_context(tc.tile_pool(name="mpool", bufs=3))
    wpool = ctx.enter_context(tc.tile_pool(name="wpool", bufs=6))
    spool = ctx.enter_context(tc.tile_pool(name="spool", bufs=3))
    ppool = ctx.enter_context(tc.tile_pool(name="ppool", bufs=3, space="PSUM"))
    apool = ctx.enter_context(tc.tile_pool(name="apool", bufs=1, space="PSUM"))
    opool = ctx.enter_context(tc.tile_pool(name="opool", bufs=1))

    # Constant weights, resident in SBUF for the whole kernel.
    wc_sb = consts.tile([P, KO + 1, HS], f16)
    nc.sync.dma_start(wc_sb[:], wc[:].rearrange("(ko ki) n -> ki ko n", ki=P))
    # Bias K-chunk lhsT: row 0 ones, rest zero -> adds wc row C (= bc) once.
    xpad = consts.tile([P, P], f16)
    nc.vector.memset(xpad[:], 0.0)
    nc.vector.memset(xpad[0:1, :], 1.0)

    # Persistent PSUM accumulators: head-pair j holds
    # G[(2 heads x 64 s), 257] = sum_n w~[n, (h,s)] * [x[n, :] | 1].
    accs = [apool.tile([P, CA], f32, tag=f"acc{j}", name=f"acc{j}")
            for j in range(4)]

    xt_r = xt[:].rearrange("(ko ki) t -> ki ko t", ki=P)
    n_tot = n_blk * n_sub

    def emit_g(wpair, xm_sb, pair, gi0):
        # reduction matmuls for a finished pair (delayed one pair so PE
        # always has the next pair's projections available — avoids
        # stalling on the exp->reduce->recip->normalize chain)
        for si in range(2):
            gi = gi0 + si
            rhs = xm_sb[:, pair * 2 + si, :]             # [128(tok), 257]
            for j in range(4):
                lhsT = wpair[:, si, j * P:(j + 1) * P]   # [128(tok), 128]
                nc.tensor.matmul(accs[j][:], lhsT, rhs,
                                 start=gi == 0, stop=gi == n_tot - 1)

    # HAM warm-up: keep the PE busy during the initial DMAs so the clock
    # gate reaches 8/8 before real work starts. Uses a memset operand so
    # it depends on no DMA and can start within ~1us of kernel entry.
    wup = consts.tile([P, HS], f16)
    nc.gpsimd.memset(wup[:], 0.0)   # gpsimd: runs parallel to DVE's xpad memsets
    for _ in range(12):
        warm = ppool.tile([P, HS], f32, tag="lg", name="warm")
        nc.tensor.matmul(warm[:], xpad[:], wup[:], start=True, stop=True)

    pending = []
    for blk in range(n_blk):
        x_sb = xpool.tile([P, KO, tt], f16)
        nc.sync.dma_start(x_sb[:], xt_r[:, :, blk * tt:(blk + 1) * tt])
        xm_sb = mpool.tile([P, n_sub, CA], f16)
        nc.sync.dma_start(
            xm_sb[:],
            xtm[blk * tt:(blk + 1) * tt, :].rearrange("(sb p) c -> p sb c", p=P))
        for pair in range(n_sub // 2):
            wpair = wpool.tile([P, 2, HS], f16)
            for si in range(2):
                sub = pair * 2 + si
                lg = ppool.tile([P, HS], f32, tag="lg")
                xk0 = x_sb[:, 0, sub * P:(sub + 1) * P]
                xk1 = x_sb[:, 1, sub * P:(sub + 1) * P]
                nc.tensor.matmul(lg[:], xpad[:], wc_sb[:, KO],
                                 start=True, stop=False)
                nc.tensor.matmul(lg[:], xk0, wc_sb[:, 0], start=False, stop=False)
                nc.tensor.matmul(lg[:], xk1, wc_sb[:, 1], start=False, stop=True)
                nc.scalar.activation(out=wpair[:, si, :], in_=lg[:],
                                     func=mybir.ActivationFunctionType.Exp)
            w4 = wpair[:].rearrange("p t (h s) -> p t h s", h=H)
            den = spool.tile([P, 2, H], f32, tag="den")
            nc.vector.tensor_reduce(out=den[:], in_=w4,
                                    axis=mybir.AxisListType.X,
                                    op=mybir.AluOpType.add)
            rec = spool.tile([P, 2, H], f16, tag="rec")
            with nc.allow_low_precision(reason="softmax denom reciprocal in f16"):
                nc.vector.reciprocal(rec[:], den[:])
            # normalize w in place on GpSimd — its software TT beats DVE's
            # slow-path broadcast handling, and it keeps DVE free to reduce
            nc.gpsimd.tensor_mul(out=w4, in0=w4,
                                 in1=rec[:, :, :, None].to_broadcast((P, 2, H, S)))
            pending.append((wpair, xm_sb, pair, blk * n_sub + pair * 2))
            if len(pending) > 3:
                emit_g(*pending.pop(0))
    while pending:
        emit_g(*pending.pop(0))

    # spread the final PSUM evictions across engines so they don't
    # serialize behind DVE's per-op DRAIN at the kernel tail
    out_sb = opool.tile([P, 4, CA], f32)
    for j in range(4):
        if j % 2 == 0:
            nc.vector.tensor_copy(out_sb[:, j, :], accs[j][:])
        else:
            nc.scalar.activation(out=out_sb[:, j, :], in_=accs[j][:],
                                 func=mybir.ActivationFunctionType.Copy)
    nc.sync.dma_start(out[:].rearrange("j p c -> p j c"), out_sb[:])


def build_bass(t_tokens=T, tt=2048, finalize=True):
    from contextlib import ExitStack
    nc = bacc.Bacc("TRN2")
    xt = nc.dram_tensor("xt", [C, t_tokens], f16, kind="ExternalInput")
    wc = nc.dram_tensor("wc", [C + P, HS], f16, kind="ExternalInput")
    xtm = nc.dram_tensor("xtm", [t_tokens, CA], f16, kind="ExternalInput")
    out = nc.dram_tensor("out", [4, P, CA], f32, kind="ExternalOutput")
    with tile.TileContext(nc) as tc:
        with ExitStack() as ctx:
            _emit(ctx, tc, xt, wc, xtm, out, t_tokens, tt)
    if finalize:
        nc.finalize()
    return nc


def make_device_weights(W_x, b_x, W_slice, b_slice, temperature):
    """Host-side weight fusion -> wc_dev [C+128, HS] f16."""
    temp = np.clip(np.asarray(temperature, np.float64).reshape(H), 0.5, 5.0)
    Wx3 = np.asarray(W_x, np.float64).reshape(C, H, D)
    Ws = np.asarray(W_slice, np.float64)
    Wc = np.einsum("chd,ds->chs", Wx3, Ws) / temp[None, :, None]
    bc = (np.asarray(b_x, np.float64).reshape(H, D) @ Ws
          + np.asarray(b_slice, np.float64)[None, :]) / temp[:, None]
    wc_dev = np.zeros((C + P, HS), np.float16)
    wc_dev[:C] = Wc.reshape(C, HS).astype(np.float16)
    wc_dev[C] = bc.reshape(HS).astype(np.float16)
    return wc_dev


def untangle(M):
    """Per-core device output [4, 128, 257] -> G [H, S, C+1] (col C = norm)."""
    M = np.asarray(M, np.float64)
    G = np.empty((H, S, CA), np.float64)
    for j in range(4):
        G[2 * j] = M[j, 0:S, :]
        G[2 * j + 1] = M[j, S:2 * S, :]
    return G


def postprocess(core_outs, W_fx, b_fx):
    Wf = np.asarray(W_fx, np.float64).reshape(C, H, D)
    bfx = np.asarray(b_fx, np.float64).reshape(H, D)
    out = np.empty((B, H, S, D), np.float32)
    for b in range(B):
        G = sum(untangle(core_outs[b * SHARDS_PER_B + q]) for q in range(SHARDS_PER_B))
        Mn = G[..., C]                      # [H, S] total softmax mass
        Q = np.einsum("hsc,chd->hsd", G[..., :C], Wf)
        res = (Q + Mn[..., None] * bfx[:, None, :]) / (Mn[..., None] + 0.01)
        out[b] = res.astype(np.float32)
    return out


def make_in_maps(x, wc_dev):
    x = np.asarray(x)
    in_maps = []
    for core in range(NCORES):
        b, q = core // SHARDS_PER_B, core % SHARDS_PER_B
        xs = x[b, q * T:(q + 1) * T, :]
        xt = np.ascontiguousarray(xs.T.astype(np.float16))
        xtm = np.empty((T, CA), np.float16)
        xtm[:, :C] = xs.astype(np.float16)
        xtm[:, C] = 1.0
        in_maps.append({"xt": xt, "wc": wc_dev, "xtm": xtm})
    return in_maps


_NC_CACHE = {}


def _get_nc():
    if "nc" not in _NC_CACHE:
        _NC_CACHE["nc"] = build_bass()
    return _NC_CACHE["nc"]


def _run(x, W_x, b_x, W_fx, b_fx, W_slice, b_slice, temperature, trace=False):
    wc_dev = make_device_weights(W_x, b_x, W_slice, b_slice, temperature)
    in_maps = make_in_maps(x, wc_dev)
    res = run_bass_kernel_spmd(_get_nc(), in_maps, core_ids=list(range(NCORES)),
                               trace=trace)
    out = postprocess([r["out"] for r in res.results], W_fx, b_fx)
    return out, res


def kernel(**inputs) -> np.ndarray:
    out, _ = _run(**inputs)
    return out


def kernel_traced(**inputs):
    out, res = _run(**inputs, trace=True)
    return out, res

